# revision 1
# baseline (speedup 1.0000x reference)
"""EnhancedDynamicChannelAttention Trainium2 kernel.

Reference computation (B=16, S=2048, C=1024, H=8, HD=128):
    q[b,h,:]   = pref[b,h]*Wq[:,0] + bq
    k          = f @ Wk.T + bk ;  v = f @ Wv.T + bv       (per head slice)
    scores     = softmax_s(q . k)                          [B,H,S]
    ctx[b,h,:] = sum_s scores * v[b,s,h,:]                 [B,H,HD]
    out        = f + broadcast_s(ctx)

Algebraic folding used here (exact up to fp reassociation):
  - softmax is shift invariant  -> the q.bk term drops entirely.
  - scores[b,h,s] = f[b,s,h,:] . qk[b,h,:]  with  qk = (pref*Wq+bq) @ Wk
  - sum_s attn = 1  ->  ctx = Wv @ (sum_s attn*f[b,s,h,:]) + bv
  So k/v are never materialized; the kernel is memory bound
  (read f once + write out once = 32 MiB per core).

Distribution: pure data parallel over batch, 2 batches per core, 8 cores.

Per-core device program (per batch b, f kept resident in SBUF):
  - DMA in f as 4 x 2MiB super tiles into one [128, 16, 1024] tile
    (s = st*512 + p*4 + t: partition p holds 4 contiguous rows per st).
  - DVE  : tmp = f * qk_bcast ; segmented reduce -> scores [128, 4, 8]
  - ACT  : E = exp(scores)  (no max-sub needed; |scores| < ~30)
  - PE   : uwf[8,1024] += E_t.T @ f_t ; sumE[8,1] += E_t.T @ ones
           (fp32, PSUM accumulation over the 16 sub tiles)
  - tail : uwf /= sumE (row scale) ; per-head PE transpose -> wfT ;
           ctx_row[1,1024] = wfT_h.T @ WvT + bv ; broadcast via DRAM.
  - adds : f += ctx_bcast split between DVE (out of place, half-tile
           grain) and GPSIMD (in place), then DMA out on the ACT ring
           (loads keep the SP ring so the two streams never queue
           behind each other).
"""

import numpy as np

B, S, C = 16, 2048, 1024
H, HD = 8, 128
N_CORES = 8
BPC = B // N_CORES          # batches per core
ST = 4                      # s-rows per partition in a super tile
P = 128
SUP = S // (P * ST)         # super tiles per batch (4)
NT = S // P                 # sub tiles per batch (16)

_CACHE = {}


def _build_program():
    import concourse.bass as bass
    import concourse.bacc as bacc
    import concourse.tile as tile
    from concourse import mybir

    f32 = mybir.dt.float32
    f32r = mybir.dt.float32r

    nc = bacc.Bacc("TRN2", debug=False, num_devices=N_CORES)
    f_in = nc.dram_tensor("features", [BPC, S, C], f32, kind="ExternalInput")
    qk_in = nc.dram_tensor("qkflat", [BPC, C], f32, kind="ExternalInput")
    wvt_in = nc.dram_tensor("wvt", [HD, HD], f32, kind="ExternalInput")
    bvf_in = nc.dram_tensor("bvflat", [1, C], f32, kind="ExternalInput")
    id8_in = nc.dram_tensor("ident8", [8, 8], f32, kind="ExternalInput")
    ones_in = nc.dram_tensor("ones128", [P, 1], f32, kind="ExternalInput")
    out_t = nc.dram_tensor("out", [BPC, S, C], f32, kind="ExternalOutput")

    with tile.TileContext(nc) as tc:
        with (
            tc.tile_pool(name="fpool", bufs=BPC) as fpool,
            tc.tile_pool(name="tmppool", bufs=2) as tmppool,
            tc.tile_pool(name="spool", bufs=2 * SUP) as spool,
            tc.tile_pool(name="small", bufs=2) as small,
            tc.tile_pool(name="singles", bufs=1) as singles,
            tc.tile_pool(name="ps1", bufs=1, space="PSUM") as ps1,
            tc.tile_pool(name="ps2", bufs=2, space="PSUM") as ps2,
            tc.tile_pool(name="dscratch", bufs=2, space="DRAM") as dscratch,
        ):
            wvt_sb = singles.tile([HD, HD], f32)
            nc.scalar.dma_start(out=wvt_sb, in_=wvt_in[:, :])
            bvf_sb = singles.tile([1, C], f32)
            nc.scalar.dma_start(out=bvf_sb, in_=bvf_in[:, :])
            id8_sb = singles.tile([8, 8], f32)
            nc.scalar.dma_start(out=id8_sb, in_=id8_in[:, :])
            ones_sb = singles.tile([P, 1], f32)
            nc.scalar.dma_start(out=ones_sb, in_=ones_in[:, :])

            # qk rows for both batches, broadcast down all partitions.
            # Loaded up front on the SP ring so batch-1 compute is never
            # queued behind batch-0 stores on the ACT ring.
            qk_bcs = []
            for b in range(BPC):
                qk_bc = small.tile([P, C], f32, tag="qkbc")
                nc.sync.dma_start(
                    out=qk_bc, in_=qk_in[b : b + 1, :].to_broadcast([P, C])
                )
                qk_bcs.append(qk_bc)

            for b in range(BPC):
                qk_bc3 = qk_bcs[b].rearrange(
                    "p (o c) -> p o c", o=1
                ).broadcast_to([P, ST, C])

                uwfA = ps2.tile([P, 512], f32, tag="uwfA")
                uwfB = ps2.tile([P, 512], f32, tag="uwfB")
                sumE = ps2.tile([8, 1], f32, tag="sumE", bufs=1)

                fview = f_in[b].rearrange("(st p t) c -> st p t c", p=P, t=ST)
                oview = out_t[b].rearrange("(st p t) c -> st p t c", p=P, t=ST)

                fb = fpool.tile([P, NT, C], f32, tag="fb")
                fb32 = fb
                for st in range(SUP):
                    for half in range(2):
                        lo = st * ST + half * (ST // 2)
                        nc.sync.dma_start(
                            out=fb[:, lo : lo + ST // 2, :],
                            in_=fview[st][:, half * (ST // 2) : (half + 1) * (ST // 2), :],
                        )

                    tmp = tmppool.tile([P, ST, C], f32, tag="tmp")
                    nc.vector.tensor_mul(
                        tmp, fb32[:, st * ST : (st + 1) * ST, :], qk_bc3
                    )
                    scores = spool.tile([P, ST, H], f32, tag="scores")
                    nc.vector.reduce_sum(
                        scores,
                        tmp.rearrange("p t (h d) -> p t h d", h=H),
                        axis=mybir.AxisListType.X,
                    )
                    E_sup = spool.tile([P, ST, H], f32, tag="esup")
                    nc.scalar.activation(
                        out=E_sup.rearrange("p t h -> p (t h)"),
                        in_=scores.rearrange("p t h -> p (t h)"),
                        func=mybir.ActivationFunctionType.Exp,
                    )

                    for t in range(ST):
                        first = st == 0 and t == 0
                        last = st == SUP - 1 and t == ST - 1
                        e_sl = E_sup[:, t, :]
                        f_sl = fb[:, st * ST + t, :]
                        nc.tensor.matmul(
                            uwfA[0:8, :], e_sl, f_sl[:, 0:512],
                            start=first, stop=last,
                        )
                        nc.tensor.matmul(
                            uwfB[0:8, :], e_sl, f_sl[:, 512:1024],
                            start=first, stop=last,
                        )
                        nc.tensor.matmul(
                            sumE, e_sl, ones_sb, start=first, stop=last
                        )

                # ---- tail: ctx_row = (diag(uwf)/sumE) @ WvT + bv ----
                recip = small.tile([8, 1], f32, tag="recip")
                nc.vector.reciprocal(recip, sumE)
                # uwf -> SBUF, normalized rows: uwf[h,:] / sumE[h]
                uwf_sb = small.tile([8, C], f32, tag="uwfsb", bufs=1)
                nc.scalar.copy(out=uwf_sb[:, 0:512], in_=uwfA[0:8, :])
                nc.scalar.copy(out=uwf_sb[:, 512:1024], in_=uwfB[0:8, :])
                nc.vector.tensor_scalar_mul(uwf_sb, uwf_sb, recip)
                # per-head PE transpose into [128, 8*8]; diagonal columns
                # (stride 9) hold wfT[d, h] = uwf[h, h*128+d] / sumE[h]
                wfT8_ps = ps1.tile([P, H * H], f32, tag="wft8")
                for h in range(H):
                    nc.tensor.transpose(
                        wfT8_ps[:, h * H : (h + 1) * H],
                        uwf_sb[:, h * HD : (h + 1) * HD],
                        id8_sb,
                    )
                wfT8_sb = small.tile([P, H * H], f32, tag="wft8sb", bufs=1)
                nc.scalar.copy(out=wfT8_sb, in_=wfT8_ps)

                ctx_ps = ps1.tile([1, C], f32, tag="ctxrow")
                for h in range(H):
                    nc.tensor.matmul(
                        ctx_ps[0:1, h * HD : (h + 1) * HD],
                        wfT8_sb[:, h * (H + 1) : h * (H + 1) + 1],
                        wvt_sb,
                        start=True,
                        stop=True,
                    )
                ctx_row = small.tile([1, C], f32, tag="ctxrowsb", bufs=1)
                nc.vector.tensor_add(ctx_row, ctx_ps, bvf_sb)
                ctx_dram = dscratch.tile([1, C], f32, tag="ctxdram")
                nc.sync.dma_start(out=ctx_dram, in_=ctx_row)
                ctx_bc = small.tile([P, C], f32, tag="ctxbc", bufs=1)
                nc.sync.dma_start(
                    out=ctx_bc, in_=ctx_dram[0:1, :].to_broadcast([P, C])
                )
                ctx_bc3 = ctx_bc.rearrange("p (o c) -> p o c", o=1).broadcast_to(
                    [P, ST, C]
                )

                # residual adds in place, split GPSIMD/DVE per super tile;
                # stores go out on the ACT HWDGE ring so next-batch loads on
                # the SP ring are not queued behind them
                ctx_bc2 = ctx_bc.rearrange("p (o c) -> p o c", o=1).broadcast_to(
                    [P, ST // 2, C]
                )
                for st in range(SUP):
                    fsl32 = fb32[:, st * ST : (st + 1) * ST, :]
                    if st % 2 == 0:
                        # last batch: DVE adds out of place (in-place DVE
                        # adds run 2-5x slower), half-super-tile grain
                        for half in range(2):
                            lo = st * ST + half * (ST // 2)
                            osl = tmppool.tile(
                                [P, ST // 2, C], f32, tag="ostage", bufs=1
                            )
                            nc.vector.tensor_add(
                                osl, fb32[:, lo : lo + ST // 2, :], ctx_bc2
                            )
                            tsl = slice(half * (ST // 2), (half + 1) * (ST // 2))
                            nc.scalar.dma_start(
                                out=oview[st][:, tsl, :], in_=osl
                            )
                    else:
                        nc.gpsimd.tensor_add(fsl32, fsl32, ctx_bc3)
                        nc.scalar.dma_start(out=oview[st], in_=fsl32)

    nc.finalize()
    return nc


def _get_program():
    if "nc" not in _CACHE:
        _CACHE["nc"] = _build_program()
    return _CACHE["nc"]


def _prep_in_maps(features, preference, Wq, bq, Wk, Wv, bv):
    f32 = np.float32
    # qk[b,h,:] = (pref[b,h]*Wq[:,0] + bq) @ Wk   -> flat [B, C]
    q = preference[:, :, None] * Wq[:, 0][None, None, :] + bq  # [B,H,HD]
    qk = np.einsum("bhe,ed->bhd", q, Wk)  # [B,H,HD]
    qkflat = np.ascontiguousarray(qk.reshape(B, C), dtype=f32)
    wvt = np.ascontiguousarray(Wv.T, dtype=f32)
    bvflat = np.ascontiguousarray(np.tile(bv, H)[None, :], dtype=f32)
    id8 = np.eye(8, dtype=f32)
    ones128 = np.ones([P, 1], dtype=f32)

    in_maps = []
    for i in range(N_CORES):
        sl = slice(i * BPC, (i + 1) * BPC)
        in_maps.append(
            {
                "features": np.ascontiguousarray(features[sl], dtype=f32),
                "qkflat": qkflat[sl],
                "wvt": wvt,
                "bvflat": bvflat,
                "ident8": id8,
                "ones128": ones128,
            }
        )
    return in_maps


def kernel(features, preference, Wq, bq, Wk, bk, Wv, bv, **_ignored):
    features = np.asarray(features, dtype=np.float32)
    preference = np.asarray(preference, dtype=np.float32)
    Wq = np.asarray(Wq, dtype=np.float32)
    bq = np.asarray(bq, dtype=np.float32)
    Wk = np.asarray(Wk, dtype=np.float32)
    Wv = np.asarray(Wv, dtype=np.float32)
    bv = np.asarray(bv, dtype=np.float32)

    from concourse.bass_utils import run_bass_kernel_spmd

    nc = _get_program()
    in_maps = _prep_in_maps(features, preference, Wq, bq, Wk, Wv, bv)
    res = run_bass_kernel_spmd(nc, in_maps, core_ids=list(range(N_CORES)))
    out = np.concatenate([r["out"] for r in res.results], axis=0)
    return out.astype(np.float32)



# revision 2
# speedup vs baseline: 1.6655x; 1.6655x over previous
"""EnhancedDynamicChannelAttention Trainium2 kernel (bf16 pipeline).

Reference computation (B=16, S=2048, C=1024, H=8, HD=128):
    q[b,h,:]   = pref[b,h]*Wq[:,0] + bq
    k          = f @ Wk.T + bk ;  v = f @ Wv.T + bv       (per head slice)
    scores     = softmax_s(q . k)                          [B,H,S]
    ctx[b,h,:] = sum_s scores * v[b,s,h,:]                 [B,H,HD]
    out        = f + broadcast_s(ctx)

Algebraic folding (exact up to fp reassociation):
  - softmax shift invariance  -> the q.bk term drops entirely.
  - scores[b,h,s] = f[b,s,h,:] . qk[b,h,:]  with  qk = (pref*Wq+bq) @ Wk
  - sum_s attn = 1  ->  ctx = Wv @ (sum_s attn*f[b,s,h,:]) + bv
  So k/v are never materialized.

Precision: the whole data path runs in bf16 (f storage, score mul,
E, PE matmul inputs, residual add, output store); reductions and the
small tail stay fp32.  Measured end-to-end rel err vs the fp64
reference is ~3e-3, comfortably inside the 2e-2 gate, and it halves
DMA bytes (16 MiB/core) while doubling DVE throughput.

Distribution: pure data parallel over batch, 2 batches per core.

Per-core device program, emitted phase by phase so each engine's
program order matches the intended schedule:
  A(b): DMA f super tiles (SP ring) ; DVE tmp=f*qk, segmented
        reduce -> scores ; ACT exp -> E (bf16) ; PE uwf/sumE PSUM accum.
  T(b): DVE recip ; ACT uwf->SBUF ; DVE row scale ; PE per-head
        transpose ; PE ctx row matmuls ; DVE +bv (bf16 out) ; PE
        ones-outer-product broadcast ctx row -> PSUM ; ACT -> bf16 SBUF.
  R(b): residual adds split Pool (in-place, early window) / DVE
        (out-of-place staging), stores on the SP ring.
Emission order A0 T0 A1 R0 T1 R1 keeps DVE busy back-to-back:
b0 scores, b1 scores, then the adds, with Pool soaking up the adds
that are ready while DVE still works on b1's scores.
"""

import numpy as np

B, S, C = 16, 2048, 1024
H, HD = 8, 128
N_CORES = 8
BPC = B // N_CORES          # batches per core
ST = 4                      # s-rows per partition in a super tile
P = 128
SUP = S // (P * ST)         # super tiles per batch (4)
NT = S // P                 # sub tiles per batch (16)

_CACHE = {}


def _build_program():
    import concourse.bass as bass
    import concourse.bacc as bacc
    import concourse.tile as tile
    from concourse import mybir

    f32 = mybir.dt.float32
    bf16 = mybir.dt.bfloat16

    nc = bacc.Bacc("TRN2", debug=False, num_devices=N_CORES)
    f_in = nc.dram_tensor("features", [BPC, S, C], bf16, kind="ExternalInput")
    qk_in = nc.dram_tensor("qkflat", [BPC, C], bf16, kind="ExternalInput")
    wvt_in = nc.dram_tensor("wvt", [HD, HD], f32, kind="ExternalInput")
    bvf_in = nc.dram_tensor("bvflat", [1, C], f32, kind="ExternalInput")
    id8_in = nc.dram_tensor("ident8", [8, 8], f32, kind="ExternalInput")
    ones_in = nc.dram_tensor("ones128", [P, 1], bf16, kind="ExternalInput")
    onesrow_in = nc.dram_tensor("onesrow", [1, P], bf16, kind="ExternalInput")
    out_t = nc.dram_tensor("out", [BPC, S, C], bf16, kind="ExternalOutput")

    with tile.TileContext(nc) as tc:
        with (
            tc.tile_pool(name="fpool", bufs=BPC) as fpool,
            tc.tile_pool(name="tmppool", bufs=2) as tmppool,
            tc.tile_pool(name="spool", bufs=2 * SUP) as spool,
            tc.tile_pool(name="small", bufs=2) as small,
            tc.tile_pool(name="ostage", bufs=4) as ostage,
            tc.tile_pool(name="singles", bufs=1) as singles,
            tc.tile_pool(name="ps_uwf", bufs=1, space="PSUM") as ps_uwf,
            tc.tile_pool(name="ps_tail", bufs=1, space="PSUM") as ps_tail,
        ):
            wvt_sb = singles.tile([HD, HD], f32)
            nc.scalar.dma_start(out=wvt_sb, in_=wvt_in[:, :])
            bvf_sb = singles.tile([1, C], f32)
            nc.scalar.dma_start(out=bvf_sb, in_=bvf_in[:, :])
            id8_sb = singles.tile([8, 8], f32)
            nc.scalar.dma_start(out=id8_sb, in_=id8_in[:, :])
            ones_sb = singles.tile([P, 1], bf16)
            nc.scalar.dma_start(out=ones_sb, in_=ones_in[:, :])
            onesrow_sb = singles.tile([1, P], bf16)
            nc.scalar.dma_start(out=onesrow_sb, in_=onesrow_in[:, :])

            # qk rows for both batches, broadcast down all partitions,
            # loaded up front on the SP ring.
            qk_bcs = []
            for b in range(BPC):
                qk_bc = small.tile([P, C], bf16, tag="qkbc")
                nc.sync.dma_start(
                    out=qk_bc, in_=qk_in[b : b + 1, :].to_broadcast([P, C])
                )
                qk_bcs.append(qk_bc)

            fbs = [None] * BPC
            uwfs = [None] * BPC
            sums = [None] * BPC
            ctxs = [None] * BPC

            def phase_scores(b):
                """Loads + scores + uwf/sumE accumulation for batch b."""
                qk_bc3 = qk_bcs[b].rearrange(
                    "p (o c) -> p o c", o=1
                ).broadcast_to([P, ST, C])

                uwfA = ps_uwf.tile([P, 512], f32, tag="uwfA")
                uwfB = ps_uwf.tile([P, 512], f32, tag="uwfB")
                sumE = ps_uwf.tile([8, 1], f32, tag="sumE")
                uwfs[b] = (uwfA, uwfB)
                sums[b] = sumE

                fview = f_in[b].rearrange("(st p t) c -> st p t c", p=P, t=ST)
                fb = fpool.tile([P, NT, C], bf16, tag="fb")
                fbs[b] = fb

                for st in range(SUP):
                    lo = st * ST
                    nc.sync.dma_start(
                        out=fb[:, lo : lo + ST, :], in_=fview[st]
                    )
                    tmp = tmppool.tile([P, ST, C], bf16, tag="tmp")
                    nc.vector.tensor_mul(
                        tmp, fb[:, lo : lo + ST, :], qk_bc3
                    )
                    scores = spool.tile([P, ST, H], f32, tag="scores")
                    nc.vector.reduce_sum(
                        scores,
                        tmp.rearrange("p t (h d) -> p t h d", h=H),
                        axis=mybir.AxisListType.X,
                    )
                    E_sup = spool.tile([P, ST, H], bf16, tag="esup")
                    nc.scalar.activation(
                        out=E_sup.rearrange("p t h -> p (t h)"),
                        in_=scores.rearrange("p t h -> p (t h)"),
                        func=mybir.ActivationFunctionType.Exp,
                    )

                    for t in range(ST):
                        first = st == 0 and t == 0
                        last = st == SUP - 1 and t == ST - 1
                        e_sl = E_sup[:, t, :]
                        f_sl = fb[:, lo + t, :]
                        nc.tensor.matmul(
                            uwfA[0:8, :], e_sl, f_sl[:, 0:512],
                            start=first, stop=last,
                        )
                        nc.tensor.matmul(
                            uwfB[0:8, :], e_sl, f_sl[:, 512:1024],
                            start=first, stop=last,
                        )
                        nc.tensor.matmul(
                            sumE, e_sl, ones_sb, start=first, stop=last
                        )

            def phase_tail(b):
                """ctx row + broadcast tile (bf16 SBUF) for batch b."""
                uwfA, uwfB = uwfs[b]
                sumE = sums[b]
                recip = small.tile([8, 1], f32, tag="recip")
                nc.vector.reciprocal(recip, sumE)
                uwf_sb = small.tile([8, C], f32, tag="uwfsb", bufs=1)
                nc.scalar.copy(out=uwf_sb[:, 0:512], in_=uwfA[0:8, :])
                nc.scalar.copy(out=uwf_sb[:, 512:1024], in_=uwfB[0:8, :])
                nc.vector.tensor_scalar_mul(uwf_sb, uwf_sb, recip)
                # per-head PE transpose into [128, 8*8]; diagonal columns
                # (stride 9) hold wfT[d, h] = uwf[h, h*128+d] / sumE[h]
                wfT8_ps = ps_tail.tile([P, H * H], f32, tag="wft8")
                for h in range(H):
                    nc.tensor.transpose(
                        wfT8_ps[:, h * H : (h + 1) * H],
                        uwf_sb[:, h * HD : (h + 1) * HD],
                        id8_sb,
                    )
                wfT8_sb = small.tile([P, H * H], f32, tag="wft8sb", bufs=1)
                nc.scalar.copy(out=wfT8_sb, in_=wfT8_ps)

                ctx_ps = ps_tail.tile([1, C], f32, tag="ctxrow")
                for h in range(H):
                    nc.tensor.matmul(
                        ctx_ps[0:1, h * HD : (h + 1) * HD],
                        wfT8_sb[:, h * (H + 1) : h * (H + 1) + 1],
                        wvt_sb,
                        start=True,
                        stop=True,
                    )
                ctx_row = small.tile([1, C], bf16, tag="ctxrowsb", bufs=1)
                nc.vector.tensor_add(ctx_row, ctx_ps, bvf_sb)
                # broadcast down partitions on the PE: ones ⊗ ctx_row
                ctx_bc_ps = ps_tail.tile([P, C], f32, tag="ctxbcps")
                for half in range(2):
                    cs = slice(half * 512, (half + 1) * 512)
                    nc.tensor.matmul(
                        ctx_bc_ps[:, cs], onesrow_sb, ctx_row[:, cs],
                        start=True, stop=True,
                    )
                ctx_bc = small.tile([P, C], bf16, tag="ctxbc")
                nc.scalar.copy(out=ctx_bc, in_=ctx_bc_ps)
                ctxs[b] = ctx_bc

            def phase_resid(b, pool_sts):
                """Residual adds + stores for batch b.  pool_sts go to the
                Pool engine in place; the rest are DVE out-of-place at
                half-super-tile grain.  Stores ride the SP ring (all loads
                are already queued ahead of them)."""
                fb = fbs[b]
                ctx_bc = ctxs[b]
                ctx_bc3 = ctx_bc.rearrange("p (o c) -> p o c", o=1).broadcast_to(
                    [P, ST, C]
                )
                ctx_bc2 = ctx_bc.rearrange("p (o c) -> p o c", o=1).broadcast_to(
                    [P, ST // 2, C]
                )
                oview = out_t[b].rearrange("(st p t) c -> st p t c", p=P, t=ST)
                for st in range(SUP):
                    if st in pool_sts:
                        for half in range(2):
                            lo = st * ST + half * (ST // 2)
                            fsl = fb[:, lo : lo + ST // 2, :]
                            nc.gpsimd.tensor_add(fsl, fsl, ctx_bc2)
                            tsl = slice(half * (ST // 2), (half + 1) * (ST // 2))
                            nc.sync.dma_start(out=oview[st][:, tsl, :], in_=fsl)
                    else:
                        for half in range(2):
                            lo = st * ST + half * (ST // 2)
                            osl = ostage.tile([P, ST // 2, C], bf16, tag="osl")
                            nc.vector.tensor_add(
                                osl, fb[:, lo : lo + ST // 2, :], ctx_bc2
                            )
                            tsl = slice(half * (ST // 2), (half + 1) * (ST // 2))
                            nc.sync.dma_start(out=oview[st][:, tsl, :], in_=osl)

            phase_scores(0)
            phase_tail(0)
            phase_scores(1)
            phase_resid(0, pool_sts=(0, 1))
            phase_tail(1)
            phase_resid(1, pool_sts=(0,))

    nc.finalize()
    return nc


def _get_program():
    if "nc" not in _CACHE:
        _CACHE["nc"] = _build_program()
    return _CACHE["nc"]


def _prep_in_maps(features, preference, Wq, bq, Wk, Wv, bv):
    import ml_dtypes

    f32 = np.float32
    bf16 = ml_dtypes.bfloat16
    # qk[b,h,:] = (pref[b,h]*Wq[:,0] + bq) @ Wk   -> flat [B, C]
    q = preference[:, :, None] * Wq[:, 0][None, None, :] + bq  # [B,H,HD]
    qk = np.einsum("bhe,ed->bhd", q, Wk)  # [B,H,HD]
    qkflat = np.ascontiguousarray(qk.reshape(B, C)).astype(bf16)
    wvt = np.ascontiguousarray(Wv.T, dtype=f32)
    bvflat = np.ascontiguousarray(np.tile(bv, H)[None, :], dtype=f32)
    id8 = np.eye(8, dtype=f32)
    ones128 = np.ones([P, 1], dtype=bf16)
    onesrow = np.ones([1, P], dtype=bf16)
    fbf = np.ascontiguousarray(features).astype(bf16)

    in_maps = []
    for i in range(N_CORES):
        sl = slice(i * BPC, (i + 1) * BPC)
        in_maps.append(
            {
                "features": fbf[sl],
                "qkflat": qkflat[sl],
                "wvt": wvt,
                "bvflat": bvflat,
                "ident8": id8,
                "ones128": ones128,
                "onesrow": onesrow,
            }
        )
    return in_maps


def kernel(features, preference, Wq, bq, Wk, bk, Wv, bv, **_ignored):
    features = np.asarray(features, dtype=np.float32)
    preference = np.asarray(preference, dtype=np.float32)
    Wq = np.asarray(Wq, dtype=np.float32)
    bq = np.asarray(bq, dtype=np.float32)
    Wk = np.asarray(Wk, dtype=np.float32)
    Wv = np.asarray(Wv, dtype=np.float32)
    bv = np.asarray(bv, dtype=np.float32)

    from concourse.bass_utils import run_bass_kernel_spmd

    nc = _get_program()
    in_maps = _prep_in_maps(features, preference, Wq, bq, Wk, Wv, bv)
    res = run_bass_kernel_spmd(nc, in_maps, core_ids=list(range(N_CORES)))
    out = np.concatenate([r["out"] for r in res.results], axis=0)
    return out.astype(np.float32)


# revision 6
# speedup vs baseline: 1.7280x; 1.0376x over previous
"""EnhancedDynamicChannelAttention Trainium2 kernel (bf16 pipeline).

Reference computation (B=16, S=2048, C=1024, H=8, HD=128):
    q[b,h,:]   = pref[b,h]*Wq[:,0] + bq
    k          = f @ Wk.T + bk ;  v = f @ Wv.T + bv       (per head slice)
    scores     = softmax_s(q . k)                          [B,H,S]
    ctx[b,h,:] = sum_s scores * v[b,s,h,:]                 [B,H,HD]
    out        = f + broadcast_s(ctx)

Algebraic folding (exact up to fp reassociation):
  - softmax shift invariance  -> the q.bk term drops entirely.
  - scores[b,h,s] = f[b,s,h,:] . qk[b,h,:]  with  qk = (pref*Wq+bq) @ Wk
  - sum_s attn = 1  ->  ctx = Wv @ (sum_s attn*f[b,s,h,:]) + bv
  So k/v are never materialized.

Precision: the data path runs in bf16 (f storage, score mul, E, PE
matmul inputs, residual add, output store); scores are fp16 (2-byte
keeps the DVE segmented reduce in its 2x perf mode, and fp16 has
enough mantissa for |scores|<~30); PSUM accumulation and the tail
stay fp32.  End-to-end rel err vs the fp64 reference ~4e-3 vs the
2e-2 gate.

Distribution: pure data parallel over batch, 2 batches per core.

Per-core phases (emitted so each engine's program order matches the
intended schedule; DVE is the critical engine):
  A(b): DMA f super tiles (SP ring; qk/const loads ride the ACT ring
        so the first f tile lands immediately) ; DVE tmp=f*qk +
        segmented reduce -> fp16 scores ; ACT exp -> bf16 E ;
        PE uwf/sumE PSUM accumulation.
  T(b): DVE recip(sumE) ; ACT copies uwf out of PSUM with the 1/sumE
        per-partition scale fused ; PE per-head transpose ; PE ctx
        matmuls into an [8,128] row-per-head tile ; DVE +bv (tiny) ->
        bf16 ; PE ones-outer-product broadcast -> PSUM ; ACT -> bf16
        SBUF tile ctx_bc.
  R(b): residual adds split Pool (in-place, soaks up the early
        window while DVE still runs scores) / DVE (out-of-place into
        staging) ; stores ride the SP ring behind all loads.
Emission: A0 T0 A1 [T1 recip] [R0 head] [T1 rest] [R0 tail] R1 keeps
DVE busy end-to-end and hides T1's PE/ACT chain under R0's adds.
"""

import numpy as np

B, S, C = 16, 2048, 1024
H, HD = 8, 128
N_CORES = 8
BPC = B // N_CORES          # batches per core
ST = 4                      # s-rows per partition in a super tile
P = 128
SUP = S // (P * ST)         # super tiles per batch (4)
NT = S // P                 # sub tiles per batch (16)

# residual-add halves handed to the Pool engine (rest go to DVE)
POOL_HALVES = {0: 3, 1: 1}

_CACHE = {}


def _build_program():
    import concourse.bass as bass
    import concourse.bacc as bacc
    import concourse.tile as tile
    from concourse import mybir

    f32 = mybir.dt.float32
    f16 = mybir.dt.float16
    bf16 = mybir.dt.bfloat16

    nc = bacc.Bacc("TRN2", debug=False, num_devices=N_CORES)
    f_in = nc.dram_tensor("features", [BPC, S, C], bf16, kind="ExternalInput")
    qk_in = nc.dram_tensor("qkflat", [BPC, C], bf16, kind="ExternalInput")
    wvt_in = nc.dram_tensor("wvt", [HD, HD], f32, kind="ExternalInput")
    bvf_in = nc.dram_tensor("bvflat", [1, C], f32, kind="ExternalInput")
    id8_in = nc.dram_tensor("ident8", [8, 8], f32, kind="ExternalInput")
    ones_in = nc.dram_tensor("ones128", [P, 1], bf16, kind="ExternalInput")
    onesrow_in = nc.dram_tensor("onesrow", [1, P], bf16, kind="ExternalInput")
    out_t = nc.dram_tensor("out", [BPC, S, C], bf16, kind="ExternalOutput")

    with tile.TileContext(nc) as tc:
        with (
            tc.tile_pool(name="fpool", bufs=BPC) as fpool,
            tc.tile_pool(name="tmppool", bufs=2) as tmppool,
            tc.tile_pool(name="spool", bufs=2 * SUP) as spool,
            tc.tile_pool(name="small", bufs=2) as small,
            tc.tile_pool(name="ostage", bufs=4) as ostage,
            tc.tile_pool(name="singles", bufs=1) as singles,
            tc.tile_pool(name="ps_uwf", bufs=1, space="PSUM") as ps_uwf,
            tc.tile_pool(name="ps_tail", bufs=1, space="PSUM") as ps_tail,
        ):
            wvt_sb = singles.tile([HD, HD], f32)
            nc.scalar.dma_start(out=wvt_sb, in_=wvt_in[:, :])
            bvf_sb = singles.tile([1, C], f32)
            nc.scalar.dma_start(out=bvf_sb, in_=bvf_in[:, :])
            id8_sb = singles.tile([8, 8], f32)
            nc.scalar.dma_start(out=id8_sb, in_=id8_in[:, :])
            ones_sb = singles.tile([P, 1], bf16)
            nc.scalar.dma_start(out=ones_sb, in_=ones_in[:, :])
            onesrow_sb = singles.tile([1, P], bf16)
            nc.scalar.dma_start(out=onesrow_sb, in_=onesrow_in[:, :])

            # qk rows for both batches, broadcast down all partitions.
            # These ride the ACT ring so the SP ring's first f load is
            # never queued behind them.
            qk_bcs = []
            for b in range(BPC):
                qk_bc = small.tile([P, C], bf16, tag="qkbc")
                nc.scalar.dma_start(
                    out=qk_bc, in_=qk_in[b : b + 1, :].to_broadcast([P, C])
                )
                qk_bcs.append(qk_bc)

            fbs = [None] * BPC
            uwfs = [None] * BPC
            sums = [None] * BPC
            recips = [None] * BPC
            ctxs = [None] * BPC

            def phase_scores(b):
                """Loads + scores + uwf/sumE accumulation for batch b."""
                qk_bc3 = qk_bcs[b].rearrange(
                    "p (o c) -> p o c", o=1
                ).broadcast_to([P, ST, C])

                uwfA = ps_uwf.tile([P, 512], f32, tag="uwfA")
                uwfB = ps_uwf.tile([P, 512], f32, tag="uwfB")
                sumE = ps_uwf.tile([8, 1], f32, tag="sumE")
                uwfs[b] = (uwfA, uwfB)
                sums[b] = sumE

                fview = f_in[b].rearrange("(st p t) c -> st p t c", p=P, t=ST)
                fb = fpool.tile([P, NT, C], bf16, tag="fb")
                fbs[b] = fb

                for st in range(SUP):
                    lo = st * ST
                    nc.sync.dma_start(
                        out=fb[:, lo : lo + ST, :], in_=fview[st]
                    )
                    tmp = tmppool.tile([P, ST, C], bf16, tag="tmp")
                    nc.vector.tensor_mul(
                        tmp, fb[:, lo : lo + ST, :], qk_bc3
                    )
                    scores = spool.tile([P, ST, H], f16, tag="scores")
                    with nc.allow_low_precision(
                        reason="fp16 scores: |s|<30, 2-byte keeps DVE 2x mode"
                    ):
                        nc.vector.reduce_sum(
                            scores,
                            tmp.rearrange("p t (h d) -> p t h d", h=H),
                            axis=mybir.AxisListType.X,
                        )
                    E_sup = spool.tile([P, ST, H], bf16, tag="esup")
                    nc.scalar.activation(
                        out=E_sup.rearrange("p t h -> p (t h)"),
                        in_=scores.rearrange("p t h -> p (t h)"),
                        func=mybir.ActivationFunctionType.Exp,
                    )

                    for t in range(ST):
                        first = st == 0 and t == 0
                        last = st == SUP - 1 and t == ST - 1
                        e_sl = E_sup[:, t, :]
                        f_sl = fb[:, lo + t, :]
                        nc.tensor.matmul(
                            uwfA[0:8, :], e_sl, f_sl[:, 0:512],
                            start=first, stop=last,
                        )
                        nc.tensor.matmul(
                            uwfB[0:8, :], e_sl, f_sl[:, 512:1024],
                            start=first, stop=last,
                        )
                        nc.tensor.matmul(
                            sumE, e_sl, ones_sb, start=first, stop=last
                        )

            def tail_recip(b):
                recip = small.tile([8, 1], f32, tag="recip")
                nc.vector.reciprocal(recip, sums[b])
                recips[b] = recip

            def tail_ctx(b):
                """ctx8 (+bv) -> broadcast bf16 SBUF tile for batch b."""
                uwfA, uwfB = uwfs[b]
                recip = recips[b]
                # PSUM -> SBUF with the 1/sumE row scale fused into the copy
                uwf_sb = small.tile([8, C], f32, tag="uwfsb", bufs=1)
                nc.scalar.activation(
                    out=uwf_sb[:, 0:512], in_=uwfA[0:8, :],
                    func=mybir.ActivationFunctionType.Copy, scale=recip,
                )
                nc.scalar.activation(
                    out=uwf_sb[:, 512:1024], in_=uwfB[0:8, :],
                    func=mybir.ActivationFunctionType.Copy, scale=recip,
                )
                # per-head PE transpose into [128, 8*8]; diagonal columns
                # (stride 9) hold wfT[d, h] = uwf[h, h*128+d] / sumE[h]
                wfT8_ps = ps_tail.tile([P, H * H], f32, tag="wft8")
                for h in range(H):
                    nc.tensor.transpose(
                        wfT8_ps[:, h * H : (h + 1) * H],
                        uwf_sb[:, h * HD : (h + 1) * HD],
                        id8_sb,
                    )
                wfT8_sb = small.tile([P, H * H], f32, tag="wft8sb", bufs=1)
                nc.scalar.copy(out=wfT8_sb, in_=wfT8_ps)

                # ctx row: ctx[0, h*HD:] = wf_h . WvT   [1, C]
                ctx_ps = ps_tail.tile([1, C], f32, tag="ctxrow")
                for h in range(H):
                    nc.tensor.matmul(
                        ctx_ps[0:1, h * HD : (h + 1) * HD],
                        wfT8_sb[:, h * (H + 1) : h * (H + 1) + 1],
                        wvt_sb,
                        start=True,
                        stop=True,
                    )
                ctx_row = small.tile([1, C], bf16, tag="ctxrowsb", bufs=1)
                nc.vector.tensor_add(ctx_row, ctx_ps, bvf_sb)
                # broadcast down partitions on the PE: ones ⊗ ctx_row
                ctx_bc_ps = ps_tail.tile([P, C], f32, tag="ctxbcps")
                for half in range(2):
                    cs = slice(half * 512, (half + 1) * 512)
                    nc.tensor.matmul(
                        ctx_bc_ps[:, cs], onesrow_sb, ctx_row[:, cs],
                        start=True, stop=True,
                    )
                ctx_bc = small.tile([P, C], bf16, tag="ctxbc")
                nc.scalar.copy(out=ctx_bc, in_=ctx_bc_ps)
                ctxs[b] = ctx_bc

            def resid_halves(b):
                """Yield (half_index, fb_slice, out_view_slice) for batch b."""
                fb = fbs[b]
                oview = out_t[b].rearrange("(st p t) c -> st p t c", p=P, t=ST)
                idx = 0
                for st in range(SUP):
                    for half in range(2):
                        lo = st * ST + half * (ST // 2)
                        tsl = slice(half * (ST // 2), (half + 1) * (ST // 2))
                        yield idx, fb[:, lo : lo + ST // 2, :], oview[st][:, tsl, :]
                        idx += 1

            def resid_pool(b):
                """Pool-engine in-place adds + stores for its share."""
                ctx_bc2 = ctxs[b].rearrange("p (o c) -> p o c", o=1).broadcast_to(
                    [P, ST // 2, C]
                )
                n = POOL_HALVES[b]
                for idx, fsl, osl in resid_halves(b):
                    if idx >= n:
                        break
                    nc.gpsimd.tensor_add(fsl, fsl, ctx_bc2)
                    nc.sync.dma_start(out=osl, in_=fsl)

            def resid_dve(b, first=None, count=None):
                """DVE out-of-place adds + stores for [first, first+count)."""
                ctx_bc2 = ctxs[b].rearrange("p (o c) -> p o c", o=1).broadcast_to(
                    [P, ST // 2, C]
                )
                n = POOL_HALVES[b]
                lo = n if first is None else first
                hi = 8 if count is None else lo + count
                for idx, fsl, osl in resid_halves(b):
                    if idx < lo or idx >= hi:
                        continue
                    ost = ostage.tile([P, ST // 2, C], bf16, tag="osl")
                    nc.vector.tensor_add(ost, fsl, ctx_bc2)
                    nc.sync.dma_start(out=osl, in_=ost)

            phase_scores(0)
            tail_recip(0)
            tail_ctx(0)
            phase_scores(1)
            tail_recip(1)
            # Pool starts b0's adds as soon as ctx0 exists (while DVE still
            # runs b1's scores); DVE picks up its b0 share afterwards, with
            # b1's tail chain hidden under it.
            resid_pool(0)
            k0 = POOL_HALVES[0]
            resid_dve(0, first=k0, count=2)
            tail_ctx(1)
            resid_dve(0, first=k0 + 2, count=8 - (k0 + 2))
            resid_pool(1)
            resid_dve(1)

    nc.finalize()
    return nc


def _get_program():
    if "nc" not in _CACHE:
        _CACHE["nc"] = _build_program()
    return _CACHE["nc"]


def _prep_in_maps(features, preference, Wq, bq, Wk, Wv, bv):
    import ml_dtypes

    f32 = np.float32
    bf16 = ml_dtypes.bfloat16
    # qk[b,h,:] = (pref[b,h]*Wq[:,0] + bq) @ Wk   -> flat [B, C]
    q = preference[:, :, None] * Wq[:, 0][None, None, :] + bq  # [B,H,HD]
    qk = np.einsum("bhe,ed->bhd", q, Wk)  # [B,H,HD]
    qkflat = np.ascontiguousarray(qk.reshape(B, C)).astype(bf16)
    wvt = np.ascontiguousarray(Wv.T, dtype=f32)
    bvflat = np.ascontiguousarray(np.tile(bv, H)[None, :], dtype=f32)
    id8 = np.eye(8, dtype=f32)
    ones128 = np.ones([P, 1], dtype=bf16)
    onesrow = np.ones([1, P], dtype=bf16)
    fbf = np.ascontiguousarray(features).astype(bf16)

    in_maps = []
    for i in range(N_CORES):
        sl = slice(i * BPC, (i + 1) * BPC)
        in_maps.append(
            {
                "features": fbf[sl],
                "qkflat": qkflat[sl],
                "wvt": wvt,
                "bvflat": bvflat,
                "ident8": id8,
                "ones128": ones128,
                "onesrow": onesrow,
            }
        )
    return in_maps


def kernel(features, preference, Wq, bq, Wk, bk, Wv, bv, **_ignored):
    features = np.asarray(features, dtype=np.float32)
    preference = np.asarray(preference, dtype=np.float32)
    Wq = np.asarray(Wq, dtype=np.float32)
    bq = np.asarray(bq, dtype=np.float32)
    Wk = np.asarray(Wk, dtype=np.float32)
    Wv = np.asarray(Wv, dtype=np.float32)
    bv = np.asarray(bv, dtype=np.float32)

    from concourse.bass_utils import run_bass_kernel_spmd

    nc = _get_program()
    in_maps = _prep_in_maps(features, preference, Wq, bq, Wk, Wv, bv)
    res = run_bass_kernel_spmd(nc, in_maps, core_ids=list(range(N_CORES)))
    out = np.concatenate([r["out"] for r in res.results], axis=0)
    return out.astype(np.float32)
